# revision 2
# baseline (speedup 1.0000x reference)
"""Trainium2 Bass kernel for BiLevelRoutingAttention (nn_BiLevelRoutingAttention_66907000537867).

Sharding: one attention head per NeuronCore (8 heads / 8 cores). Each core:
  phase 1: qkv projection for its head (f32 matmuls), producing
           q/k channel-major (bf16), v in padded-image layout (bf16, for the
           lepe depthwise conv folded into the output projection) and v in
           pixel-major layout with a ones column (bf16, for attn@V + softmax
           denominators).
  phase 2: per region (49): QK^T -> exp (ScalarE, scale fused) -> attn@V.
           Softmax normalization: denominators (ones-column matmul output) are
           scatter-DMA'd across partitions, reciprocal'd on DVE, gathered back,
           broadcast via a K=1 matmul and multiplied in.
  phase 3: output projection with lepe folded in: 10 accumulating taps
           (9 shifted dwconv taps with host-folded diag(lepe_w) @ w_o + the
           attention tap), row-tiled 4x across PE via tile_position.

Host: window-ordering of pixels, region routing (top-k is metadata; the mean
commutes exactly with the linear qkv layer), per-head weight slicing, final
sum of per-core partials + constant bias row.
"""

import numpy as np
import ml_dtypes

import concourse.bass as bass
import concourse.bacc as bacc
import concourse.mybir as mybir
import concourse.tile as tile
from concourse.tile import add_dep_helper
from concourse.bass_utils import run_bass_kernel_spmd

F32 = mybir.dt.float32
BF16 = mybir.dt.bfloat16
AF = mybir.ActivationFunctionType

DIM, QK, HEADS, NWIN, TOPK = 256, 256, 8, 7, 4
H = W = 112
P2 = NWIN * NWIN          # 49 regions
W2 = 256                  # pixels per region (16x16)
NPIX = H * W              # 12544
HD = 32                   # per-head dim
SCALE = QK ** (-0.5)      # 1/16
NT = 25                   # pixel tiles: 24x512 + 1x256
PW = 114                  # padded image width
N3 = 448                  # phase-3 pixel tile (4 image rows)

_cache = {}


def _tile_w(t):
    return 512 if t < 24 else 256


def _build(top_idx, debug=False):
    nc = bacc.Bacc()
    xT_d = nc.declare_dram_parameter("xT", [DIM, NPIX], F32, isOutput=False)
    wqkv_d = nc.declare_dram_parameter("wqkv", [DIM, 96], F32, isOutput=False)
    bqkv_d = nc.declare_dram_parameter("bqkv", [96, 1], F32, isOutput=False)
    wt_d = nc.declare_dram_parameter("wt", [HD, 20 * 128], BF16, isOutput=False)
    out_d = nc.declare_dram_parameter("out", [DIM, NPIX], F32, isOutput=True)
    dsc_d = nc.dram_tensor("dscratch", [25, 512], F32)
    dsc2_d = nc.dram_tensor("dscratch2", [25, 512], F32)

    with tile.TileContext(nc) as tc, tc.tile_pool(name="persist", bufs=1) as persist:
        # ---- persistent SBUF ----
        w_sb = persist.tile([128, 192], F32)          # qkv weights, 2 cin chunks
        bqkv_sb = persist.tile([96, 1], F32)
        qk_sb = persist.tile([64, NPIX], BF16)        # rows 0-31 q, 32-63 k
        kx_sb = persist.tile([32, NPIX], BF16)        # k copy at partition base 0
        v_aug = persist.tile([128, 98, 34], BF16)     # pixel-major v + ones col 32 (34-stride keeps rows 4B-aligned)
        v_pix = persist.tile([128, 98 * 32], BF16)    # contiguous transpose staging
        v_pad = persist.tile([128, PW * PW], BF16)    # 4 bands of padded-image v
        out_u = persist.tile([32, NPIX], F32)         # unnormalized attn out (ch-major)
        out_cm = persist.tile([128, NPIX], BF16)      # 4 bands of normalized attn out
        wt_sb = persist.tile([128, 20 * 128], BF16)   # 4 bands of proj stationaries
        ones_sb = persist.tile([1, 32], F32)

        nc.sync.dma_start(out=w_sb[:, 0:96], in_=wqkv_d[0:128, :])
        nc.sync.dma_start(out=w_sb[:, 96:192], in_=wqkv_d[128:256, :])
        nc.sync.dma_start(out=bqkv_sb, in_=bqkv_d[:, :])
        for b in range(4):
            nc.sync.dma_start(out=wt_sb[32 * b:32 * b + 32, :], in_=wt_d[:, :])
        nc.vector.memset(ones_sb, 1.0)
        nc.vector.memset(v_aug[:, :, 32:33], 1.0)
        # zero the padded border (whole tensor; interiors overwritten)
        nc.gpsimd.memset(v_pad, 0.0)

        v_pad_v = v_pad.rearrange("p (r c) -> p r c", c=PW)

        # ---- phase 1: qkv projection ----
        with (
            tc.tile_pool(name="xt", bufs=3) as xtp,
            tc.tile_pool(name="vstage", bufs=4) as vsp,
            tc.tile_pool(name="qkv_ps", bufs=2, space="PSUM") as qkvps,
            tc.tile_pool(name="dum_ps", bufs=2, space="PSUM") as dumps,
        ):
            # this walrus only allows ONE sync wait on a self-loading f32
            # matmul: pre-observe each DMA semaphore with a tiny dummy matmul
            # ordered before the real one so the real matmul needs <=1 wait.
            def observe(aps, dum_pool):
                dum = dum_pool.tile([1, 1], F32, tag="dum")
                last = None
                for ap in aps:
                    d = nc.tensor.matmul(dum, ap[0:1, 0:1], ap[0:1, 0:1],
                                         start=True, stop=True)
                    if last is not None:
                        add_dep_helper(d.ins, last.ins, sync=False)
                    last = d
                return last

            for t in range(NT):
                w = _tile_w(t)
                n0 = 512 * t
                xt0 = xtp.tile([128, w], F32, tag="xt0")
                xt1 = xtp.tile([128, w], F32, tag="xt1")
                nc.sync.dma_start(out=xt0, in_=xT_d[0:128, n0:n0 + w])
                nc.sync.dma_start(out=xt1, in_=xT_d[128:256, n0:n0 + w])
                obs = observe([w_sb, xt0, xt1] if t == 0 else [xt0, xt1], dumps)
                ps = qkvps.tile([96, w], F32, tag="qkv")
                m1 = nc.tensor.matmul(ps, w_sb[:, 0:96], xt0, start=True, stop=False)
                add_dep_helper(m1.ins, obs.ins, sync=False)
                nc.tensor.matmul(ps, w_sb[:, 96:192], xt1, start=False, stop=True)
                # q+k evacuation with bias, f32 -> bf16 (single DVE op keeps the
                # psum-WAR fan-in on one engine semaphore)
                nc.vector.tensor_scalar_add(qk_sb[:, n0:n0 + w], ps[0:64, :],
                                            bqkv_sb[0:64, 0:1])
                # v evacuation to bf16 staging (partitions 64-95)
                vs = vsp.tile([96, w], BF16, tag="vs")
                nc.vector.tensor_scalar_add(vs[64:96, :], ps[64:96, :],
                                            bqkv_sb[64:96, 0:1])
                # v -> padded image layout (band 2 = partitions 64-95)
                vsv = vs.rearrange("p (a b c) -> p a b c", b=16, c=16)
                for wi in range(w // 256):
                    win = 2 * t + wi
                    wr, wc = divmod(win, NWIN)
                    nc.vector.tensor_copy(
                        v_pad_v[64:96, 16 * wr + 1:16 * wr + 17,
                                16 * wc + 1:16 * wc + 17],
                        vsv[64:96, wi, :, :])
                # v -> pixel-major staging via DMA xbar transpose; one
                # 128-col tile per call, 64B-aligned contiguous destinations
                for ci in range(w // 128):
                    c = 4 * t + ci
                    nc.sync.dma_start_transpose(
                        out=v_pix[:, 32 * c:32 * (c + 1)],
                        in_=vs[64:96, 128 * ci:128 * (ci + 1)])

            # interleave v into the 34-stride v_aug slots (col 32 stays ones)
            nc.vector.tensor_copy(v_aug[:, :, 0:32],
                                  v_pix.rearrange("p (c j) -> p c j", j=32))
            # k copy to partition base 0; v_pad band replication
            nc.sync.dma_start(out=kx_sb[:, :], in_=qk_sb[32:64, :])
            for b in (0, 1, 3):
                nc.sync.dma_start(out=v_pad[32 * b:32 * b + 32, :],
                                  in_=v_pad[64:96, :])

        tc.strict_bb_all_engine_barrier()

        # ---- phase 2: attention ----
        with (
            tc.tile_pool(name="attnT_ps", bufs=2, space="PSUM") as atps,
            tc.tile_pool(name="outT_ps", bufs=1, space="PSUM") as otps,
            tc.tile_pool(name="bc_ps", bufs=1, space="PSUM") as bcps,
            tc.tile_pool(name="expT", bufs=4) as expp,
            tc.tile_pool(name="dstage", bufs=4) as dsp,
            tc.tile_pool(name="dum2_ps", bufs=2, space="PSUM") as dum2,
        ):
            def observe2(aps):
                dum = dum2.tile([1, 1], F32, tag="dum2")
                last = None
                for ap in aps:
                    d = nc.tensor.matmul(dum, ap[0:1, 0:1], ap[0:1, 0:1],
                                         start=True, stop=True)
                    if last is not None:
                        add_dep_helper(d.ins, last.ins, sync=False)
                    last = d
                return last

            dst = None
            for r in range(P2):
                chunks = [2 * g + jj for g in top_idx[r] for jj in (0, 1)]
                q_ap = qk_sb[0:32, W2 * r:W2 * (r + 1)]
                exs = []
                for half in range(2):
                    at = atps.tile([128, 1024], F32, tag="at")
                    for j4 in range(4):
                        c = chunks[4 * half + j4]
                        nc.tensor.matmul(at[:, 256 * j4:256 * (j4 + 1)],
                                         kx_sb[:, 128 * c:128 * (c + 1)],
                                         q_ap, start=True, stop=True)
                    ex = expp.tile([128, 1024], BF16, tag="ex")
                    nc.scalar.activation(ex, at, AF.Exp, scale=SCALE)
                    exs.append(ex)
                outT = otps.tile([33, W2], F32, tag="ot")
                for j in range(8):
                    nc.tensor.matmul(outT, v_aug[:, chunks[j], 0:33],
                                     exs[j // 4][:, 256 * (j % 4):256 * (j % 4 + 1)],
                                     start=(j == 0), stop=(j == 7))
                nc.vector.tensor_copy(out_u[:, W2 * r:W2 * (r + 1)], outT[0:32, :])
                # stash denominators (psum row 32 -> sbuf partition 32)
                if r % 2 == 0:
                    dst = dsp.tile([33, 512], F32, tag="dst")
                nc.vector.tensor_copy(dst[32:33, 256 * (r % 2):256 * (r % 2) + 256],
                                      outT[32:33, :])
                if r % 2 == 1 or r == P2 - 1:
                    pw = 512 if r % 2 == 1 else 256
                    p0 = 512 * (r // 2)
                    pi = r // 2
                    nj = pw // 128
                    # scatter across partitions (via DRAM), reciprocal, gather back
                    nc.sync.dma_start(out=dsc_d[pi, 0:pw], in_=dst[32:33, 0:pw])
                    dp = dsp.tile([128, 4], F32, tag="dp")
                    nc.sync.dma_start(
                        out=dp[:, 0:nj],
                        in_=dsc_d[pi, 0:pw].rearrange("(j q) -> q j", q=128))
                    dpr = dsp.tile([128, 4], F32, tag="dpr")
                    nc.vector.reciprocal(dpr[:, 0:nj], dp[:, 0:nj])
                    nc.sync.dma_start(
                        out=dsc2_d[pi, 0:pw].rearrange("(j q) -> q j", q=128),
                        in_=dpr[:, 0:nj])
                    dr = dsp.tile([1, 512], F32, tag="dr")
                    nc.sync.dma_start(out=dr[0:1, 0:pw], in_=dsc2_d[pi, 0:pw])
                    bc = bcps.tile([32, 512], F32, tag="bc")
                    obs = observe2([dr])
                    mb = nc.tensor.matmul(bc[:, 0:pw], ones_sb[:, :], dr[0:1, 0:pw],
                                          start=True, stop=True)
                    add_dep_helper(mb.ins, obs.ins, sync=False)
                    nc.vector.tensor_mul(out_cm[0:32, p0:p0 + pw],
                                         out_u[:, p0:p0 + pw], bc[:, 0:pw])

            # out_cm band replication
            for b in (1, 2, 3):
                nc.sync.dma_start(out=out_cm[32 * b:32 * b + 32, :],
                                  in_=out_cm[0:32, :])

        tc.strict_bb_all_engine_barrier()

        # ---- phase 3: output projection + folded lepe ----
        out_cm_w = out_cm.rearrange("p (w a b) -> p w a b", a=16, b=16)
        with (
            tc.tile_pool(name="o_ps", bufs=4, space="PSUM") as ops,
            tc.tile_pool(name="osb", bufs=4) as osbp,
        ):
            for n in range(28):
                b = n % 4
                sl = slice(32 * b, 32 * b + 32)
                tp = (32 * b, 0)
                for hh in range(2):
                    acc = ops.tile([128, N3], F32, tag="acc")
                    for t in range(9):
                        dy, dx = divmod(t, 3)
                        rhs = v_pad_v[sl, 4 * n + dy:4 * n + dy + 4, dx:dx + 112]
                        nc.tensor.matmul(acc, wt_sb[sl, 128 * (2 * t + hh):
                                                    128 * (2 * t + hh + 1)],
                                         rhs, start=(t == 0), stop=False,
                                         tile_position=tp)
                    wr_, py0 = n // 4, (4 * n) % 16
                    rhs = out_cm_w[sl, 7 * wr_:7 * wr_ + 7, py0:py0 + 4, :]
                    rhs = rhs.rearrange("p w a b -> p a w b")
                    nc.tensor.matmul(acc, wt_sb[sl, 128 * (18 + hh):128 * (19 + hh)],
                                     rhs, start=False, stop=True, tile_position=tp)
                    ev = osbp.tile([128, N3], F32, tag="ev")
                    if n % 2 == 0:
                        nc.vector.tensor_copy(ev, acc)
                    else:
                        nc.scalar.copy(ev, acc)
                    nc.sync.dma_start(
                        out=out_d[128 * hh:128 * (hh + 1), N3 * n:N3 * (n + 1)],
                        in_=ev)

        if debug:
            tc.strict_bb_all_engine_barrier()
            dbg = {
                "dbg_qk": qk_sb, "dbg_kx": kx_sb, "dbg_vaug": v_aug,
                "dbg_vpad": v_pad, "dbg_outu": out_u, "dbg_outcm": out_cm,
            }
            for name, t in dbg.items():
                sh = [t.shape[0], int(np.prod(t.shape[1:]))]
                d = nc.declare_dram_parameter(name, sh, t.dtype, isOutput=True)
                nc.sync.dma_start(out=d[:, :], in_=t.rearrange(
                    "p ... -> p (...)") if len(t.shape) > 2 else t[:, :])
            dd = nc.declare_dram_parameter("dbg_dr", [25, 512], F32, isOutput=True)
            nc.sync.dma_start(out=dd[:, :], in_=dsc2_d[:, :])
    nc.compile()
    return nc


def _host_prep(x, w_qkv, b_qkv):
    xw = x.reshape(NWIN, 16, NWIN, 16, DIM).transpose(0, 2, 1, 3, 4)
    xw = np.ascontiguousarray(xw.reshape(NPIX, DIM))
    xT = np.ascontiguousarray(xw.T)
    xmean = xw.reshape(P2, W2, DIM).mean(1)
    q_win = xmean @ w_qkv[:, :QK] + b_qkv[:QK]
    k_win = xmean @ w_qkv[:, QK:2 * QK] + b_qkv[QK:2 * QK]
    logit = (q_win * SCALE) @ k_win.T
    top_idx = np.argsort(-logit, axis=-1, kind="stable")[:, :TOPK]
    return xT, top_idx


def _in_maps(x, w_qkv, b_qkv, w_o, lepe_w):
    xT, top_idx = _host_prep(x[0], w_qkv, b_qkv)
    lw = lepe_w[:, :, 0, :]  # [3,3,256]
    maps = []
    for h in range(HEADS):
        sl = slice(h * HD, (h + 1) * HD)
        wqkv_h = np.concatenate(
            [w_qkv[:, :QK][:, sl], w_qkv[:, QK:2 * QK][:, sl],
             w_qkv[:, 2 * QK:][:, sl]], axis=1)
        bqkv_h = np.concatenate(
            [b_qkv[:QK][sl], b_qkv[QK:2 * QK][sl], b_qkv[2 * QK:][sl]])
        w_o_h = w_o[sl, :]  # [32, 256]
        blocks = []
        for t in range(9):
            dy, dx = divmod(t, 3)
            wt_full = lw[dy, dx, sl][:, None] * w_o_h
            blocks += [wt_full[:, 0:128], wt_full[:, 128:256]]
        blocks += [w_o_h[:, 0:128], w_o_h[:, 128:256]]
        wt_h = np.concatenate(blocks, axis=1).astype(ml_dtypes.bfloat16)
        maps.append({
            "xT": xT,
            "wqkv": np.ascontiguousarray(wqkv_h),
            "bqkv": np.ascontiguousarray(bqkv_h[:, None]),
            "wt": np.ascontiguousarray(wt_h),
        })
    return maps, top_idx


def kernel(x, w_qkv, b_qkv, w_o, b_o, lepe_w, lepe_b):
    x = np.asarray(x, np.float32)
    w_qkv = np.asarray(w_qkv, np.float32)
    b_qkv = np.asarray(b_qkv, np.float32)
    w_o = np.asarray(w_o, np.float32)
    b_o = np.asarray(b_o, np.float32)
    lepe_w = np.asarray(lepe_w, np.float32)
    lepe_b = np.asarray(lepe_b, np.float32)

    maps, top_idx = _in_maps(x, w_qkv, b_qkv, w_o, lepe_w)
    key = top_idx.tobytes()
    if key not in _cache:
        _cache[key] = _build(top_idx)
    nc = _cache[key]
    global _last_build
    _last_build = (nc, maps)

    res = run_bass_kernel_spmd(nc, maps, list(range(HEADS))).results
    total = np.zeros((DIM, NPIX), np.float32)
    for h in range(HEADS):
        total += np.asarray(res[h]["out"], np.float32)
    b_all = lepe_b @ w_o + b_o
    out = total.T + b_all
    return out.reshape(1, H, W, DIM).astype(np.float32)



# revision 22
# speedup vs baseline: 1.1501x; 1.1501x over previous
"""Trainium2 Bass kernel for BiLevelRoutingAttention (nn_BiLevelRoutingAttention_66907000537867).

Sharding: one attention head per NeuronCore (8 heads / 8 cores).

Per-core pipeline (phases overlap via tile dependencies, no barriers):
  phase 1 (qkv): bf16 xT tiles -> PE qkv matmul -> DVE evac to qkv0
     (q|k|v channel-major, window-major pixels). q/k replicated to 4
     partition bands via sbuf-sbuf DMA (for 4x row-packed QK^T).
     v additionally: gpsimd window->image copy into pix4 center band,
     row-shifted band replicas via DMA, and a batched DMA xbar transpose
     into pixel-major v_aug (with a ones column for softmax denominators).
  phase 2 (attention, per region in readiness order): QK^T 4x row-packed
     (K=32 strips), exp on ScalarE (scale fused), attn@V accumulating
     [33,256] (33rd row = denominators). Denominators: DVE copy ->
     DRAM scatter -> DVE reciprocal [128,x] -> DRAM -> partition-broadcast
     gather -> DVE normalize-multiply into attn_sb (window-major bf16).
  phase 3 (per window row, interleaved into phase 2): lepe depthwise conv
     as 3 dx-group matmuls over row-shifted v bands (diag-block weights),
     4 n-tiles col-packed; projection = 2 concurrent row-strip taps
     (lepe tap + attn tap) per 128-out-chunk; f32 out DMA (image-major).

Host: window-ordering of pixels, region routing (top-k metadata baked into
the build), per-head weight slicing/packing, final sum of per-core partials
+ constant bias row.
"""

import numpy as np
import ml_dtypes

import concourse.bass as bass
import concourse.bacc as bacc
import concourse.mybir as mybir
import concourse.tile as tile
from concourse.bass_utils import run_bass_kernel_spmd

F32 = mybir.dt.float32
BF16 = mybir.dt.bfloat16
AF = mybir.ActivationFunctionType

DIM, QK, HEADS, NWIN, TOPK = 256, 256, 8, 7, 4
H = W = 112
P2 = NWIN * NWIN          # 49 regions
W2 = 256                  # pixels per region (16x16)
NPIX = H * W              # 12544
HD = 32                   # per-head dim
SCALE = QK ** (-0.5)      # 1/16
PW = 114                  # padded image width
PH = 114                  # padded image height
NT1 = 25                  # phase-1 tiles: 24x512 + 1x256
N3 = 448                  # phase-3 n-tile (4 image rows)

_cache = {}
_last_build = None


def _tile_w(t):
    return 512 if t < 24 else 256


def _region_order(top_idx):
    """Process regions in data-readiness order (phase-1 tile index)."""
    def ready(r):
        tiles = [r // 2]
        for g in top_idx[r]:
            tiles.append(min(int(g) // 2, NT1 - 1))
        return max(tiles)
    return sorted(range(P2), key=lambda r: (ready(r), r))


def _build(top_idx, debug=False, stage="all"):
    S = ("p1", "qk", "av", "den", "norm", "all").index(stage)
    nc = bacc.Bacc()
    xT_d = nc.declare_dram_parameter("xT", [DIM, NPIX], BF16, isOutput=False)
    wqkv_d = nc.declare_dram_parameter("wqkv", [128, 192], BF16, isOutput=False)
    bqkv_d = nc.declare_dram_parameter("bqkv", [96, 1], F32, isOutput=False)
    wdx_d = nc.declare_dram_parameter("wdx", [128, 96], BF16, isOutput=False)
    wo4_d = nc.declare_dram_parameter("wo4", [128, 256], BF16, isOutput=False)
    out_d = nc.declare_dram_parameter("out", [DIM, NPIX], F32, isOutput=True)
    dsc_d = nc.dram_tensor("dscratch", [25, 512], F32)
    dsc2_d = nc.dram_tensor("dscratch2", [25, 512], F32)

    order = _region_order(top_idx)

    with tile.TileContext(nc) as tc, tc.tile_pool(name="persist", bufs=1) as persist:
        # ---- persistent SBUF ----
        w_sb = persist.tile([128, 192], BF16)
        bqkv_sb = persist.tile([96, 1], F32)
        wdx_sb = persist.tile([128, 96], BF16)
        wo4_sb = persist.tile([128, 256], BF16)
        qkv0 = persist.tile([96, NPIX], BF16)     # q 0-31 | k 32-63 | v 64-95
        q4 = persist.tile([64, NPIX], BF16)       # q band 1 (parts 32-63)
        k4 = persist.tile([32, NPIX], BF16)       # k band 0 (parts 0-31)
        v_aug = persist.tile([128, 98, 48], BF16)  # pixel-major v + ones col 32
        pix4 = persist.tile([128, PH * PW], BF16)  # v row-shift bands 32-127
        attn_sb = persist.tile([128, NPIX], BF16)  # normalized attn, 4 bands

        nc.sync.dma_start(out=w_sb, in_=wqkv_d[:, :])
        nc.sync.dma_start(out=bqkv_sb, in_=bqkv_d[:, :])
        nc.sync.dma_start(out=wdx_sb, in_=wdx_d[:, :])
        nc.sync.dma_start(out=wo4_sb, in_=wo4_d[:, :])
        nc.gpsimd.memset(pix4, 0.0)
        nc.vector.memset(v_aug[:, :, 32:33], 1.0)

        pix4_v = pix4.rearrange("p (r c) -> p r c", c=PW)
        qkv0_w = qkv0.rearrange("p (r y x) -> p r y x", y=16, x=16)
        attn_w = attn_sb.rearrange("p (r y x) -> p r y x", y=16, x=16)

        # ---- phase 1: qkv projection ----
        with (
            tc.tile_pool(name="xt", bufs=3) as xtp,
            tc.tile_pool(name="qkv_ps", bufs=2, space="PSUM") as qkvps,
        ):
            for t in range(NT1):
                w = _tile_w(t)
                n0 = 512 * t
                xt0 = xtp.tile([128, w], BF16, tag="xt0")
                xt1 = xtp.tile([128, w], BF16, tag="xt1")
                nc.sync.dma_start(out=xt0, in_=xT_d[0:128, n0:n0 + w])
                nc.sync.dma_start(out=xt1, in_=xT_d[128:256, n0:n0 + w])
                ps = qkvps.tile([96, w], F32, tag="qkv")
                nc.tensor.matmul(ps, w_sb[:, 0:96], xt0, start=True, stop=False)
                nc.tensor.matmul(ps, w_sb[:, 96:192], xt1, start=False, stop=True)
                # single evac op: q,k,v (+bias) psum -> bf16 sbuf
                nc.vector.tensor_scalar_add(qkv0[0:96, n0:n0 + w], ps,
                                            bqkv_sb[0:96, 0:1])
                # q/k band replication (sbuf->sbuf DMA, crosses partitions)
                nc.sync.dma_start(out=q4[32:64, n0:n0 + w],
                                  in_=qkv0[0:32, n0:n0 + w])
                nc.sync.dma_start(out=k4[0:32, n0:n0 + w],
                                  in_=qkv0[32:64, n0:n0 + w])
                # v -> pix4 center band (dy=1, parts 64-95), image layout,
                # on gpsimd; split tile's windows by image window-row
                wins = range(n0 // 256, n0 // 256 + w // 256)
                by_wr = {}
                for win in wins:
                    by_wr.setdefault(win // NWIN, []).append(win)
                for wr, ws in by_wr.items():
                    wc0 = ws[0] % NWIN
                    nw = len(ws)
                    src = qkv0_w[64:96, ws[0]:ws[0] + nw, :, :]
                    dst = pix4_v[64:96, 16 * wr:16 * wr + 16,
                                 16 * wc0 + 1:16 * wc0 + 1 + 16 * nw]
                    dst = dst.rearrange("p y (r x) -> p r y x", x=16)
                    nc.gpsimd.tensor_copy(dst, src)
                # v -> pixel-major (batched xbar transpose, 8 chunks/tile)
                c0 = n0 // 128
                nc.sync.dma_start_transpose(
                    out=v_aug[:, c0:c0 + w // 128, 0:32],
                    in_=qkv0[64:96, n0:n0 + w])

            # row-shifted v band replicas (contiguous sbuf->sbuf)
            nc.sync.dma_start(out=pix4[32:64, PW:PH * PW],
                              in_=pix4[64:96, 0:(PH - 1) * PW])
            nc.sync.dma_start(out=pix4[96:128, 0:(PH - 1) * PW],
                              in_=pix4[64:96, PW:PH * PW])

        # ---- phases 2+3 ----
        emitted_wr = set()
        normed = set()

        def emit_phase3(wr, p3ps, p3s, osb):
            # lepe: 3 dx-group matmuls, 4 n-tiles col-packed (distinct
            # partition slices of one bank -- legal; row strips are not)
            dxps = p3ps.tile([128, N3], F32, tag="dx")
            for dx in range(3):
                for cp in range(4):
                    nt = 4 * wr + cp
                    rhs = pix4_v[0:128, 4 * nt:4 * nt + 4, dx:dx + 112]
                    nc.tensor.matmul(dxps[32 * cp:32 * cp + 32, :],
                                     wdx_sb[:, 32 * dx:32 * dx + 32], rhs,
                                     start=(dx == 0), stop=(dx == 2),
                                     tile_position=(0, 32 * cp))
            lep = p3s.tile([128, N3], BF16, tag="lep")
            nc.vector.tensor_copy(lep, dxps)
            for cp in range(4):
                nt = 4 * wr + cp
                ys = 4 * (nt % 4)
                arhs = attn_w[32 * cp:32 * cp + 32,
                              NWIN * wr:NWIN * wr + NWIN, ys:ys + 4, :]
                arhs = arhs.rearrange("p w y x -> p y w x")
                for hh in range(2):
                    # both taps on row strip 32*cp: strictly sequential
                    # accumulation into the same bank (concurrent row
                    # strips to one bank crash the device)
                    pp = p3ps.tile([128, N3], F32, tag="pp")
                    nc.tensor.matmul(pp, wo4_sb[32 * cp:32 * cp + 32,
                                                128 * hh:128 * hh + 128],
                                     lep[32 * cp:32 * cp + 32, :],
                                     start=True, stop=False,
                                     tile_position=(32 * cp, 0))
                    nc.tensor.matmul(pp, wo4_sb[32 * cp:32 * cp + 32,
                                                128 * hh:128 * hh + 128],
                                     arhs, start=False, stop=True,
                                     tile_position=(32 * cp, 0))
                    ev = osb.tile([128, N3], F32, tag="ev")
                    nc.vector.tensor_copy(ev, pp)
                    nc.sync.dma_start(
                        out=out_d[128 * hh:128 * hh + 128,
                                  N3 * nt:N3 * nt + N3],
                        in_=ev)

        with (
            tc.tile_pool(name="at_ps", bufs=2, space="PSUM") as atp,
            tc.tile_pool(name="p1_ps", bufs=2, space="PSUM") as p1p,
            tc.tile_pool(name="p3_ps", bufs=1, space="PSUM") as p3ps,
            tc.tile_pool(name="exp", bufs=4) as expp,
            tc.tile_pool(name="dst", bufs=2) as dsp,
            tc.tile_pool(name="dr32", bufs=2) as drp,
            tc.tile_pool(name="p3s", bufs=2) as p3s,
            tc.tile_pool(name="osb", bufs=4) as osb,
        ):
            P1 = None
            pair = []  # regions in current pair
            for idx, r in enumerate(order):
                if S < 1:
                    break
                chunks = [2 * g + jj for g in top_idx[r] for jj in (0, 1)]
                # column block of chunk i within its exp tile: strips 0/1
                # alternate and must land in different PSUM banks
                colmap = (0, 512, 256, 768)
                exs = []
                for g in range(2):
                    at = atp.tile([128, 1024], F32, tag="at")
                    for i in range(4):
                        c = chunks[4 * g + i]
                        ii = i % 2
                        lhs = (qkv0[32:64, 128 * c:128 * c + 128] if ii == 1
                               else k4[0:32, 128 * c:128 * c + 128])
                        rhs = (qkv0[0:32, W2 * r:W2 * r + W2] if ii == 0
                               else q4[32:64, W2 * r:W2 * r + W2])
                        nc.tensor.matmul(at[:, colmap[i]:colmap[i] + 256],
                                         lhs, rhs, start=True, stop=True,
                                         tile_position=(32 * ii, 0))
                    ex = expp.tile([128, 1024], BF16, tag="ex")
                    nc.scalar.activation(ex, at, AF.Exp, scale=SCALE)
                    exs.append(ex)
                # attn @ V (+ denominator row 32)
                if S < 2:
                    continue
                half = len(pair)
                if half == 0:
                    P1 = p1p.tile([33, 512], F32, tag="p1")
                col = 256 * half
                for j in range(8):
                    cb = colmap[j % 4]
                    nc.tensor.matmul(P1[0:33, col:col + 256],
                                     v_aug[:, chunks[j], 0:33],
                                     exs[j // 4][:, cb:cb + 256],
                                     start=(j == 0), stop=(j == 7),
                                     tile_position=(0, 0))
                pair.append(r)
                if S < 3:
                    pair = []
                    continue
                if len(pair) == 2 or idx == P2 - 1:
                    pw = 256 * len(pair)
                    pi = idx // 2
                    nj = pw // 128
                    dst = dsp.tile([33, 512], F32, tag="dst")
                    nc.vector.tensor_copy(dst[32:33, 0:pw], P1[32:33, 0:pw])
                    nc.sync.dma_start(out=dsc_d[pi, 0:pw], in_=dst[32:33, 0:pw])
                    dp = dsp.tile([128, 4], F32, tag="dp")
                    nc.sync.dma_start(
                        out=dp[:, 0:nj],
                        in_=dsc_d[pi, 0:pw].rearrange("(j q) -> q j", q=128))
                    dpr = dsp.tile([128, 4], F32, tag="dpr")
                    nc.vector.reciprocal(dpr[:, 0:nj], dp[:, 0:nj])
                    nc.sync.dma_start(
                        out=dsc2_d[pi, 0:pw].rearrange("(j q) -> q j", q=128),
                        in_=dpr[:, 0:nj])
                    dr = drp.tile([32, 512], F32, tag="dr")
                    nc.sync.dma_start(
                        out=dr[:, 0:pw],
                        in_=dsc2_d[pi:pi + 1, 0:pw].to_broadcast([32, pw]))
                    # normalize into attn_sb (window-major, per region)
                    if S < 4:
                        pair = []
                        continue
                    for hh2, rr in enumerate(pair):
                        nc.vector.tensor_mul(
                            attn_sb[0:32, W2 * rr:W2 * rr + W2],
                            P1[0:32, 256 * hh2:256 * hh2 + 256],
                            dr[:, 256 * hh2:256 * hh2 + 256])
                        for b in (1, 2, 3):
                            nc.sync.dma_start(
                                out=attn_sb[32 * b:32 * b + 32,
                                            W2 * rr:W2 * rr + W2],
                                in_=attn_sb[0:32, W2 * rr:W2 * rr + W2])
                        normed.add(rr)
                    pair = []
                    if S < 5:
                        continue
                    for wr in range(NWIN):
                        if wr not in emitted_wr and all(
                                (NWIN * wr + k) in normed for k in range(NWIN)):
                            emitted_wr.add(wr)
                            emit_phase3(wr, p3ps, p3s, osb)

        if debug:
            tc.strict_bb_all_engine_barrier()
            dbg = {
                "dbg_qkv0": qkv0, "dbg_q4": q4, "dbg_k4": k4,
                "dbg_vaug": v_aug, "dbg_pix4": pix4,
            }
            if S >= 4:
                dbg["dbg_attn"] = attn_sb
            for name, t in dbg.items():
                sh = [t.shape[0], int(np.prod(t.shape[1:]))]
                d = nc.declare_dram_parameter(name, sh, t.dtype, isOutput=True)
                nc.sync.dma_start(out=d[:, :], in_=t.rearrange(
                    "p ... -> p (...)") if len(t.shape) > 2 else t[:, :])
    nc.compile()
    return nc


def _host_prep(x, w_qkv, b_qkv):
    xw = x.reshape(NWIN, 16, NWIN, 16, DIM).transpose(0, 2, 1, 3, 4)
    xw = np.ascontiguousarray(xw.reshape(NPIX, DIM))
    xT = np.ascontiguousarray(xw.T).astype(ml_dtypes.bfloat16)
    xmean = xw.reshape(P2, W2, DIM).mean(1)
    q_win = xmean @ w_qkv[:, :QK] + b_qkv[:QK]
    k_win = xmean @ w_qkv[:, QK:2 * QK] + b_qkv[QK:2 * QK]
    logit = (q_win * SCALE) @ k_win.T
    top_idx = np.argsort(-logit, axis=-1, kind="stable")[:, :TOPK]
    return xT, top_idx


def _in_maps(x, w_qkv, b_qkv, w_o, lepe_w):
    xT, top_idx = _host_prep(x[0], w_qkv, b_qkv)
    lw = lepe_w[:, :, 0, :]  # [3,3,256]
    maps = []
    for h in range(HEADS):
        sl = slice(h * HD, (h + 1) * HD)
        wqkv_h = np.concatenate(
            [w_qkv[:, :QK][:, sl], w_qkv[:, QK:2 * QK][:, sl],
             w_qkv[:, 2 * QK:][:, sl]], axis=1)  # [256, 96]
        wqkv_p = np.concatenate([wqkv_h[0:128, :], wqkv_h[128:256, :]],
                                axis=1).astype(ml_dtypes.bfloat16)
        bqkv_h = np.concatenate(
            [b_qkv[:QK][sl], b_qkv[QK:2 * QK][sl], b_qkv[2 * QK:][sl]])
        wdx = np.zeros((128, 96), np.float32)
        for dx in range(3):
            for dy in range(3):
                for c in range(HD):
                    wdx[32 + 32 * dy + c, 32 * dx + c] = lw[dy, dx, h * HD + c]
        wo4 = np.zeros((128, 256), np.float32)
        for b in range(4):
            wo4[32 * b:32 * b + 32, :] = w_o[sl, :]
        maps.append({
            "xT": xT,
            "wqkv": np.ascontiguousarray(wqkv_p),
            "bqkv": np.ascontiguousarray(bqkv_h[:, None].astype(np.float32)),
            "wdx": wdx.astype(ml_dtypes.bfloat16),
            "wo4": wo4.astype(ml_dtypes.bfloat16),
        })
    return maps, top_idx


def kernel(x, w_qkv, b_qkv, w_o, b_o, lepe_w, lepe_b):
    x = np.asarray(x, np.float32)
    w_qkv = np.asarray(w_qkv, np.float32)
    b_qkv = np.asarray(b_qkv, np.float32)
    w_o = np.asarray(w_o, np.float32)
    b_o = np.asarray(b_o, np.float32)
    lepe_w = np.asarray(lepe_w, np.float32)
    lepe_b = np.asarray(lepe_b, np.float32)

    maps, top_idx = _in_maps(x, w_qkv, b_qkv, w_o, lepe_w)
    key = top_idx.tobytes()
    if key not in _cache:
        _cache[key] = _build(top_idx)
    nc = _cache[key]
    global _last_build
    _last_build = (nc, maps)

    res = run_bass_kernel_spmd(nc, maps, list(range(HEADS))).results
    total = np.zeros((DIM, NPIX), np.float32)
    for h in range(HEADS):
        total += np.asarray(res[h]["out"], np.float32)
    b_all = lepe_b @ w_o + b_o
    out = total.T + b_all
    return out.reshape(1, H, W, DIM).astype(np.float32)


# revision 27
# speedup vs baseline: 1.6381x; 1.4243x over previous
"""Trainium2 Bass kernel for BiLevelRoutingAttention (nn_BiLevelRoutingAttention_66907000537867).

Sharding: one attention head per NeuronCore (8 heads / 8 cores).

Per-core pipeline (phases overlap via tile dependencies, no barriers):
  phase 1 (qkv): bf16 xT tiles -> PE qkv matmul -> DVE evac to qkv0
     (q|k|v channel-major, window-major pixels). q/k replicated to 4
     partition bands via sbuf-sbuf DMA (for 4x row-packed QK^T).
     v additionally: gpsimd window->image copy into pix4 center band,
     row-shifted band replicas via DMA, and a batched DMA xbar transpose
     into pixel-major v_aug (with a ones column for softmax denominators).
  phase 2 (attention, per region in readiness order): QK^T 4x row-packed
     (K=32 strips), exp on ScalarE (scale fused), attn@V accumulating
     [33,256] (33rd row = denominators). Denominators: DVE copy ->
     DRAM scatter -> DVE reciprocal [128,x] -> DRAM -> partition-broadcast
     gather -> DVE normalize-multiply into attn_sb (window-major bf16).
  phase 3 (per window row, interleaved into phase 2): lepe depthwise conv
     as 3 dx-group matmuls over row-shifted v bands (diag-block weights),
     4 n-tiles col-packed; projection = 2 concurrent row-strip taps
     (lepe tap + attn tap) per 128-out-chunk; f32 out DMA (image-major).

Host: window-ordering of pixels, region routing (top-k metadata baked into
the build), per-head weight slicing/packing, final sum of per-core partials
+ constant bias row.
"""

import numpy as np
import ml_dtypes

import concourse.bass as bass
import concourse.bacc as bacc
import concourse.mybir as mybir
import concourse.tile as tile
from concourse.bass_utils import run_bass_kernel_spmd

F32 = mybir.dt.float32
BF16 = mybir.dt.bfloat16
AF = mybir.ActivationFunctionType

DIM, QK, HEADS, NWIN, TOPK = 256, 256, 8, 7, 4
H = W = 112
P2 = NWIN * NWIN          # 49 regions
W2 = 256                  # pixels per region (16x16)
NPIX = H * W              # 12544
HD = 32                   # per-head dim
SCALE = QK ** (-0.5)      # 1/16
PW = 114                  # padded image width
PH = 114                  # padded image height
NT1 = 25                  # phase-1 tiles: 24x512 + 1x256
N3 = 448                  # phase-3 n-tile (4 image rows)

_cache = {}
_last_build = None


def _tile_w(t):
    return 512 if t < 24 else 256


def _region_order(top_idx):
    """Process regions in data-readiness order (phase-1 tile index)."""
    def ready(r):
        tiles = [r // 2]
        for g in top_idx[r]:
            tiles.append(min(int(g) // 2, NT1 - 1))
        return max(tiles)
    return sorted(range(P2), key=lambda r: (ready(r), r))


def _build(top_idx, debug=False, stage="all"):
    S = ("p1", "qk", "av", "den", "norm", "all").index(stage)
    nc = bacc.Bacc()
    xT_d = nc.declare_dram_parameter("xT", [DIM, NPIX], BF16, isOutput=False)
    wqkv_d = nc.declare_dram_parameter("wqkv", [128, 192], BF16, isOutput=False)
    bqkv_d = nc.declare_dram_parameter("bqkv", [96, 1], F32, isOutput=False)
    wdx_d = nc.declare_dram_parameter("wdx", [128, 96], BF16, isOutput=False)
    wo4_d = nc.declare_dram_parameter("wo4", [128, 256], BF16, isOutput=False)
    out_d = nc.declare_dram_parameter("out", [DIM, NPIX], F32, isOutput=True)
    dsc_d = nc.dram_tensor("dscratch", [25, 512], F32)
    dsc2_d = nc.dram_tensor("dscratch2", [25, 512], F32)

    order = _region_order(top_idx)

    with tile.TileContext(nc) as tc, tc.tile_pool(name="persist", bufs=1) as persist:
        # ---- persistent SBUF ----
        w_sb = persist.tile([128, 192], BF16)
        bqkv_sb = persist.tile([96, 1], F32)
        wdx_sb = persist.tile([128, 96], BF16)
        wo4_sb = persist.tile([128, 256], BF16)
        qkv0 = persist.tile([96, NPIX], BF16)     # q 0-31 | k 32-63 | v 64-95
        q4 = persist.tile([64, NPIX], BF16)       # q band 1 (parts 32-63)
        k4 = persist.tile([32, NPIX], BF16)       # k band 0 (parts 0-31)
        v_aug = persist.tile([128, 98, 48], BF16)  # pixel-major v + ones col 32
        pix4 = persist.tile([128, PH * PW], BF16)  # v row-shift bands 32-127
        attn_sb = persist.tile([128, NPIX], BF16)  # normalized attn, 4 bands

        nc.sync.dma_start(out=w_sb, in_=wqkv_d[:, :])
        nc.sync.dma_start(out=bqkv_sb, in_=bqkv_d[:, :])
        nc.sync.dma_start(out=wdx_sb, in_=wdx_d[:, :])
        nc.sync.dma_start(out=wo4_sb, in_=wo4_d[:, :])
        nc.gpsimd.memset(pix4, 0.0)
        nc.vector.memset(v_aug[:, :, 32:33], 1.0)

        pix4_v = pix4.rearrange("p (r c) -> p r c", c=PW)
        qkv0_w = qkv0.rearrange("p (r y x) -> p r y x", y=16, x=16)
        attn_w = attn_sb.rearrange("p (r y x) -> p r y x", y=16, x=16)

        # ---- phase 1: qkv projection ----
        with (
            tc.tile_pool(name="xt", bufs=3) as xtp,
            tc.tile_pool(name="qkv_ps", bufs=2, space="PSUM") as qkvps,
        ):
            for t in range(NT1):
                w = _tile_w(t)
                n0 = 512 * t
                xt0 = xtp.tile([128, w], BF16, tag="xt0")
                xt1 = xtp.tile([128, w], BF16, tag="xt1")
                nc.sync.dma_start(out=xt0, in_=xT_d[0:128, n0:n0 + w])
                nc.sync.dma_start(out=xt1, in_=xT_d[128:256, n0:n0 + w])
                ps = qkvps.tile([96, w], F32, tag="qkv")
                nc.tensor.matmul(ps, w_sb[:, 0:96], xt0, start=True, stop=False)
                nc.tensor.matmul(ps, w_sb[:, 96:192], xt1, start=False, stop=True)
                # single evac op: q,k,v (+bias) psum -> bf16 sbuf
                nc.vector.tensor_scalar_add(qkv0[0:96, n0:n0 + w], ps,
                                            bqkv_sb[0:96, 0:1])
                # q/k band replication (sbuf->sbuf DMA, crosses partitions)
                # on the ACT HWDGE queue: keeps the SP queue a pure xT
                # prefetch stream (no dependent waits blocking loads)
                nc.scalar.dma_start(out=q4[32:64, n0:n0 + w],
                                    in_=qkv0[0:32, n0:n0 + w])
                nc.scalar.dma_start(out=k4[0:32, n0:n0 + w],
                                    in_=qkv0[32:64, n0:n0 + w])
                # v -> pix4 center band (dy=1, parts 64-95), image layout,
                # on gpsimd; split tile's windows by image window-row
                wins = range(n0 // 256, n0 // 256 + w // 256)
                by_wr = {}
                for win in wins:
                    by_wr.setdefault(win // NWIN, []).append(win)
                for wr, ws in by_wr.items():
                    wc0 = ws[0] % NWIN
                    nw = len(ws)
                    src = qkv0_w[64:96, ws[0]:ws[0] + nw, :, :]
                    dst = pix4_v[64:96, 16 * wr:16 * wr + 16,
                                 16 * wc0 + 1:16 * wc0 + 1 + 16 * nw]
                    dst = dst.rearrange("p y (r x) -> p r y x", x=16)
                    nc.gpsimd.tensor_copy(dst, src)
            # v -> pixel-major: ONE batched xbar transpose (a transpose
            # serializes against every sbuf-sbuf DMA, so emit exactly one)
            nc.sync.dma_start_transpose(out=v_aug[:, :, 0:32],
                                        in_=qkv0[64:96, :])
            # row-shifted v band replicas (contiguous sbuf->sbuf)
            nc.sync.dma_start(out=pix4[32:64, PW:PH * PW],
                              in_=pix4[64:96, 0:(PH - 1) * PW])
            nc.sync.dma_start(out=pix4[96:128, 0:(PH - 1) * PW],
                              in_=pix4[64:96, PW:PH * PW])

        # ---- phases 2+3 ----
        emitted_wr = set()
        normed = set()

        def emit_phase3(wr, p3ps, p3s, osb):
            # lepe: 3 dx-group matmuls, 4 n-tiles col-packed (distinct
            # partition slices of one bank -- legal; row strips are not)
            dxps = p3ps.tile([128, N3], F32, tag="dx")
            for dx in range(3):
                for cp in range(4):
                    nt = 4 * wr + cp
                    rhs = pix4_v[0:128, 4 * nt:4 * nt + 4, dx:dx + 112]
                    nc.tensor.matmul(dxps[32 * cp:32 * cp + 32, :],
                                     wdx_sb[:, 32 * dx:32 * dx + 32], rhs,
                                     start=(dx == 0), stop=(dx == 2),
                                     tile_position=(0, 32 * cp))
            lep = p3s.tile([128, N3], BF16, tag="lep")
            nc.vector.tensor_copy(lep, dxps)
            for cp in range(4):
                nt = 4 * wr + cp
                ys = 4 * (nt % 4)
                arhs = attn_w[32 * cp:32 * cp + 32,
                              NWIN * wr:NWIN * wr + NWIN, ys:ys + 4, :]
                arhs = arhs.rearrange("p w y x -> p y w x")
                for hh in range(2):
                    # both taps on row strip 32*cp: strictly sequential
                    # accumulation into the same bank (concurrent row
                    # strips to one bank crash the device)
                    pp = p3ps.tile([128, N3], F32, tag="pp")
                    nc.tensor.matmul(pp, wo4_sb[32 * cp:32 * cp + 32,
                                                128 * hh:128 * hh + 128],
                                     lep[32 * cp:32 * cp + 32, :],
                                     start=True, stop=False,
                                     tile_position=(32 * cp, 0))
                    nc.tensor.matmul(pp, wo4_sb[32 * cp:32 * cp + 32,
                                                128 * hh:128 * hh + 128],
                                     arhs, start=False, stop=True,
                                     tile_position=(32 * cp, 0))
                    ev = osb.tile([128, N3], F32, tag="ev")
                    nc.vector.tensor_copy(ev, pp)
                    nc.gpsimd.dma_start(
                        out=out_d[128 * hh:128 * hh + 128,
                                  N3 * nt:N3 * nt + N3],
                        in_=ev)

        # column block of chunk i within its exp tile: strips 0/1 alternate
        # and must land in different PSUM banks
        colmap = (0, 512, 256, 768)

        with (
            tc.tile_pool(name="at_ps", bufs=2, space="PSUM") as atp,
            tc.tile_pool(name="p1_ps", bufs=2, space="PSUM") as p1p,
            tc.tile_pool(name="p3_ps", bufs=1, space="PSUM") as p3ps,
            tc.tile_pool(name="exp", bufs=4) as expp,
            tc.tile_pool(name="dau", bufs=3) as dsp,
            tc.tile_pool(name="dr32", bufs=3) as drp,
            tc.tile_pool(name="p3s", bufs=2) as p3s,
            tc.tile_pool(name="osb", bufs=4) as osb,
        ):
            def emit_qk(r, chunks):
                exs = []
                for g in range(2):
                    at = atp.tile([128, 1024], F32, tag="at")
                    for i in range(4):
                        c = chunks[4 * g + i]
                        ii = i % 2
                        lhs = (qkv0[32:64, 128 * c:128 * c + 128] if ii == 1
                               else k4[0:32, 128 * c:128 * c + 128])
                        rhs = (qkv0[0:32, W2 * r:W2 * r + W2] if ii == 0
                               else q4[32:64, W2 * r:W2 * r + W2])
                        nc.tensor.matmul(at[:, colmap[i]:colmap[i] + 256],
                                         lhs, rhs, start=True, stop=True,
                                         tile_position=(32 * ii, 0))
                    ex = expp.tile([128, 1024], BF16, tag="ex")
                    nc.scalar.activation(ex, at, AF.Exp, scale=SCALE)
                    exs.append(ex)
                return exs

            pairs = []          # finalized pair records
            cur = None          # open pair {P1, regs}

            def emit_av(r, chunks, exs):
                nonlocal cur
                if cur is None:
                    cur = {"P1": p1p.tile([33, 512], F32, tag="p1", name="P1"),
                           "regs": []}
                col = 256 * len(cur["regs"])
                for j in range(8):
                    cb = colmap[j % 4]
                    nc.tensor.matmul(cur["P1"][0:33, col:col + 256],
                                     v_aug[:, chunks[j], 0:33],
                                     exs[j // 4][:, cb:cb + 256],
                                     start=(j == 0), stop=(j == 7),
                                     tile_position=(0, 0))
                cur["regs"].append(r)

            def stage_a(p):
                # dens+attn_u off psum in one DVE op; scatter dens to DRAM
                pw, pi = p["pw"], p["pi"]
                dau = dsp.tile([33, 512], F32, tag="dau")
                nc.vector.tensor_copy(dau[:, 0:pw], p["P1"][:, 0:pw])
                nc.sync.dma_start(out=dsc_d[pi, 0:pw], in_=dau[32:33, 0:pw])
                dp = dsp.tile([128, 4], F32, tag="dp")
                nc.sync.dma_start(
                    out=dp[:, 0:pw // 128],
                    in_=dsc_d[pi, 0:pw].rearrange("(j q) -> q j", q=128))
                p["dau"], p["dp"] = dau, dp

            def stage_b(p):
                pw, pi, nj = p["pw"], p["pi"], p["pw"] // 128
                dpr = dsp.tile([128, 4], F32, tag="dpr")
                nc.vector.reciprocal(dpr[:, 0:nj], p["dp"][:, 0:nj])
                nc.sync.dma_start(
                    out=dsc2_d[pi, 0:pw].rearrange("(j q) -> q j", q=128),
                    in_=dpr[:, 0:nj])
                dr = drp.tile([32, 512], F32, tag="dr")
                nc.sync.dma_start(
                    out=dr[:, 0:pw],
                    in_=dsc2_d[pi:pi + 1, 0:pw].to_broadcast([32, pw]))
                p["dr"] = dr

            def stage_c(p):
                if S < 4:
                    return
                for hh2, rr in enumerate(p["regs"]):
                    nc.vector.tensor_mul(
                        attn_sb[0:32, W2 * rr:W2 * rr + W2],
                        p["dau"][0:32, 256 * hh2:256 * hh2 + 256],
                        p["dr"][:, 256 * hh2:256 * hh2 + 256])
                    for b in (1, 2, 3):
                        nc.sync.dma_start(
                            out=attn_sb[32 * b:32 * b + 32,
                                        W2 * rr:W2 * rr + W2],
                            in_=attn_sb[0:32, W2 * rr:W2 * rr + W2])
                    normed.add(rr)

            wrq = []  # window rows ready for phase 3, one event of lag

            def pump():
                K = len(pairs) - 1
                if S < 3:
                    return
                stage_a(pairs[K])
                if K >= 1:
                    stage_b(pairs[K - 1])
                if K >= 2:
                    stage_c(pairs[K - 2])
                    if S >= 5:
                        for wr in list(wrq):
                            wrq.remove(wr)
                            emit_phase3(wr, p3ps, p3s, osb)
                        for wr in range(NWIN):
                            if wr not in emitted_wr and all(
                                    (NWIN * wr + k) in normed
                                    for k in range(NWIN)):
                                emitted_wr.add(wr)
                                wrq.append(wr)

            def finalize():
                nonlocal cur
                cur["pw"] = 256 * len(cur["regs"])
                cur["pi"] = len(pairs)
                pairs.append(cur)
                cur = None
                pump()

            pend = None
            for idx, r in enumerate(order):
                if S < 1:
                    break
                chunks = [2 * g + jj for g in top_idx[r] for jj in (0, 1)]
                exs = emit_qk(r, chunks)
                if S < 2:
                    continue
                if pend is not None:
                    emit_av(*pend)
                    if len(cur["regs"]) == 2:
                        finalize()
                pend = (r, chunks, exs)
            if S >= 2 and pend is not None:
                emit_av(*pend)
                finalize()
            if S >= 3:
                # drain the pipeline with virtual events
                K = len(pairs)
                stage_b(pairs[K - 1])
                stage_c(pairs[K - 2])
                stage_c(pairs[K - 1])
                if S >= 5:
                    for wr in list(wrq):
                        emit_phase3(wr, p3ps, p3s, osb)
                    for wr in range(NWIN):
                        if wr not in emitted_wr:
                            emitted_wr.add(wr)
                            emit_phase3(wr, p3ps, p3s, osb)

        if debug:
            tc.strict_bb_all_engine_barrier()
            dbg = {
                "dbg_qkv0": qkv0, "dbg_q4": q4, "dbg_k4": k4,
                "dbg_vaug": v_aug, "dbg_pix4": pix4,
            }
            if S >= 4:
                dbg["dbg_attn"] = attn_sb
            for name, t in dbg.items():
                sh = [t.shape[0], int(np.prod(t.shape[1:]))]
                d = nc.declare_dram_parameter(name, sh, t.dtype, isOutput=True)
                nc.sync.dma_start(out=d[:, :], in_=t.rearrange(
                    "p ... -> p (...)") if len(t.shape) > 2 else t[:, :])
    nc.compile()
    return nc


def _host_prep(x, w_qkv, b_qkv):
    xw = x.reshape(NWIN, 16, NWIN, 16, DIM).transpose(0, 2, 1, 3, 4)
    xw = np.ascontiguousarray(xw.reshape(NPIX, DIM))
    xT = np.ascontiguousarray(xw.T).astype(ml_dtypes.bfloat16)
    xmean = xw.reshape(P2, W2, DIM).mean(1)
    q_win = xmean @ w_qkv[:, :QK] + b_qkv[:QK]
    k_win = xmean @ w_qkv[:, QK:2 * QK] + b_qkv[QK:2 * QK]
    logit = (q_win * SCALE) @ k_win.T
    top_idx = np.argsort(-logit, axis=-1, kind="stable")[:, :TOPK]
    return xT, top_idx


def _in_maps(x, w_qkv, b_qkv, w_o, lepe_w):
    xT, top_idx = _host_prep(x[0], w_qkv, b_qkv)
    lw = lepe_w[:, :, 0, :]  # [3,3,256]
    maps = []
    for h in range(HEADS):
        sl = slice(h * HD, (h + 1) * HD)
        wqkv_h = np.concatenate(
            [w_qkv[:, :QK][:, sl], w_qkv[:, QK:2 * QK][:, sl],
             w_qkv[:, 2 * QK:][:, sl]], axis=1)  # [256, 96]
        wqkv_p = np.concatenate([wqkv_h[0:128, :], wqkv_h[128:256, :]],
                                axis=1).astype(ml_dtypes.bfloat16)
        bqkv_h = np.concatenate(
            [b_qkv[:QK][sl], b_qkv[QK:2 * QK][sl], b_qkv[2 * QK:][sl]])
        wdx = np.zeros((128, 96), np.float32)
        for dx in range(3):
            for dy in range(3):
                for c in range(HD):
                    wdx[32 + 32 * dy + c, 32 * dx + c] = lw[dy, dx, h * HD + c]
        wo4 = np.zeros((128, 256), np.float32)
        for b in range(4):
            wo4[32 * b:32 * b + 32, :] = w_o[sl, :]
        maps.append({
            "xT": xT,
            "wqkv": np.ascontiguousarray(wqkv_p),
            "bqkv": np.ascontiguousarray(bqkv_h[:, None].astype(np.float32)),
            "wdx": wdx.astype(ml_dtypes.bfloat16),
            "wo4": wo4.astype(ml_dtypes.bfloat16),
        })
    return maps, top_idx


def kernel(x, w_qkv, b_qkv, w_o, b_o, lepe_w, lepe_b):
    x = np.asarray(x, np.float32)
    w_qkv = np.asarray(w_qkv, np.float32)
    b_qkv = np.asarray(b_qkv, np.float32)
    w_o = np.asarray(w_o, np.float32)
    b_o = np.asarray(b_o, np.float32)
    lepe_w = np.asarray(lepe_w, np.float32)
    lepe_b = np.asarray(lepe_b, np.float32)

    maps, top_idx = _in_maps(x, w_qkv, b_qkv, w_o, lepe_w)
    key = top_idx.tobytes()
    if key not in _cache:
        _cache[key] = _build(top_idx)
    nc = _cache[key]
    global _last_build
    _last_build = (nc, maps)

    res = run_bass_kernel_spmd(nc, maps, list(range(HEADS))).results
    total = np.zeros((DIM, NPIX), np.float32)
    for h in range(HEADS):
        total += np.asarray(res[h]["out"], np.float32)
    b_all = lepe_b @ w_o + b_o
    out = total.T + b_all
    return out.reshape(1, H, W, DIM).astype(np.float32)
